# revision 68
# baseline (speedup 1.0000x reference)
"""Additive (Bahdanau) attention on 8 Trainium2 NeuronCores.

Full inputs in, full output out. Data-parallel over batch with
input-value-specialized programs: the masked softmax makes key positions
k >= valid_len irrelevant (they get -1e9 before exp), so each compiled
program only computes feats/tanh up to a per-pair key extent T (rounded
to 8 -- fp32r matmuls need an even moving free count on real HW, and
f32r with a partial-partition destination is rejected outright, hence
plain f32 for the projection / output matmuls). Batches are sorted by
valid_len, paired (2 batches pack the 128 SBUF partitions: rows 0-63 =
batch even's 64 hidden dims, 64-127 = batch odd), and pairs are
assigned to cores big-with-small. Cores are grouped into <= MAXPROGS
classes; each class compiles one program parameterized by (T0, T1) =
slot extents (T0 >= T1 so the serial epilogue hangs off the small
pair). Candidate partitions are scored by TimelineSim (max modeled
class time, minimized also over the PREP1_MS virtual-schedule knob).

The feats pipeline runs in bf16 by default (BFMIN=600): tanh cost on
ACT is dtype-independent, but DVE adds get the 4x perf mode and PE
matmuls run 1 cyc/row at any width (f32r only reaches that at moving
width >= 256). Output error ~2e-3 absmax-relative vs the 2e-2 gate.

Per (batch b, query j):  scores[j, k] = sum_h wv[h] * tanh(qW[b,j,h] + kW[b,k,h])
Layout on chip: h on partitions, k on free dim.
  - DVE tensor_scalar_add broadcasts qW[:, j] onto kW (2x fp32 / 4x bf16)
  - ACT does one big in-place tanh over a j-chunk at a time (the
    bottleneck engine: ~0.83ns/elem/lane, no fast dtype mode)
  - PE contracts with wv via a sliding-window stationary matrix,
    accumulating all 64 j's into one [128, T] PSUM bank; the additive
    -1e9 mask matmul CLOSES the group (start comes from j=0) so the
    valid_lens DMA stays off the startup critical path
  - ACT Exp (PSUM->SBUF) with fused accum_out row-sum, DVE reciprocal
  - PE transposes attn, accumulated matmul vs values, scale rows by
    1/sum, DMA out. Pair 0's exp/epilogue is interleaved into pair 1's
    main loop so ACT never stalls on PE's matmul tail.

Scheduling constraint: matmul/transpose instructions lower to a fused
LDWEIGHTS (S3_LW) slot that fits only ONE semaphore wait. So every PE
operand's last writer is kept on DVE or ACT (per-engine sems merge into
one monotonic wait); DMA-landed constants get a small DVE copy first,
and one priming PE op observes the DVE clock early.
"""

import os
import sys

import numpy as np

for _p in ("/opt/trn_rl_repo", "/root/.axon_site/_ro/trn_rl_repo"):
    if os.path.isdir(_p) and _p not in sys.path:
        sys.path.append(_p)

B, Q, K, H, V = 32, 64, 512, 64, 64
NCORES = 8
BPC = B // NCORES          # batches per core
NPAIR = BPC // 2           # batch pairs per core
NEG = -1e9
MAXPROGS = int(os.environ.get("MAXPROGS", "8"))

_NC_CACHE = {}


def _chunks(total, role):
    """j-chunk schedule summing to `total` (=Q).

    Head taper: chunk i+1's DVE adds must finish inside chunk i's tanh
    plus the accumulated slack, or ACT stalls. With adds ~0.8x tanh per j
    the sizes can grow ~1.25x per step from a small start.
    """
    env = os.environ.get("CH0" if role == "head" else "CH1")
    if env:
        c = [int(x) for x in env.split(",")]
    elif role == "head":
        c = [2, 4, 8, 16, 17, 17]
    else:  # tail: big first, tiny last so the epilogue starts asap
        c = [16, 16, 12, 8, 6, 4, 2]
    assert sum(c) == total, c
    return c


def build_nc(T0=512, T1=512, mm_relaxed=None, prep1_ms=None, act2=None,
             ch1=None, ch0=None, kdma_merge=True):
    """One-core program for pair extents (T0, T1), T0 >= T1."""
    if mm_relaxed is None:
        mm_relaxed = os.environ.get("MM_RELAXED", "1") == "1"
    if prep1_ms is None:
        prep1_ms = float(os.environ.get("PREP1_MS", "0.02"))
    if act2 is None:
        act2 = os.environ.get("ACT2", "0") == "1"
    import concourse.bass as bass  # noqa: F401
    from concourse import mybir
    from concourse import tile
    from concourse.masks import make_identity

    f32 = mybir.dt.float32
    f32r = mybir.dt.float32r if mm_relaxed else mybir.dt.float32
    bf16 = mybir.dt.bfloat16
    i32 = mybir.dt.int32
    Tanh = mybir.ActivationFunctionType.Tanh
    Exp = mybir.ActivationFunctionType.Exp
    Copy = mybir.ActivationFunctionType.Copy

    from concourse import bacc

    nc = bacc.Bacc("TRN2", target_bir_lowering=False, debug=False, num_devices=NCORES)

    queries_d = nc.dram_tensor("queries", [BPC, Q, H], f32, kind="ExternalInput")
    keys_d = nc.dram_tensor("keys", [BPC, K, H], f32, kind="ExternalInput")
    values_d = nc.dram_tensor("values", [BPC, K, V], f32, kind="ExternalInput")
    vlens_d = nc.dram_tensor("valid_lens", [BPC, 1], i32, kind="ExternalInput")
    Wq_d = nc.dram_tensor("Wq", [H, H], f32, kind="ExternalInput")
    Wk_d = nc.dram_tensor("Wk", [H, H], f32, kind="ExternalInput")
    wv_d = nc.dram_tensor("wv", [H, 1], f32r, kind="ExternalInput")
    out_d = nc.dram_tensor("out", [BPC, Q, V], f32, kind="ExternalOutput")

    # fp32r matmuls require an even moving free count (and 8B-aligned
    # patterns); round extents to multiples of 8.
    Ts = [min(K, (int(t) + 7) // 8 * 8) for t in (T0, T1)]
    assert Ts[0] >= Ts[1] >= 8 and Ts[0] <= K
    # bf16 feats pipeline: required for T < 256 (f32r matmuls are 4x slower
    # below 256 moving width); optional above it (BFMIN lowers the cutoff)
    BFMIN = int(os.environ.get("BFMIN", "600"))
    BF = [t < BFMIN or t < 256 for t in Ts]
    CKs = [(t + 127) // 128 for t in Ts]    # 128-wide key chunks loaded/used
    CHUNKS_BY_PAIR = [
        list(ch0) if ch0 is not None else _chunks(Q, "head"),
        list(ch1) if ch1 is not None else _chunks(Q, "tail"),
    ]
    FMAXs = [max(c) for c in CHUNKS_BY_PAIR]
    # F pools must fit SBUF (192KB/partition) alongside ~40KB of other
    # tiles; shrink the ring when the extents are large.
    fbytes = sum(
        FMAXs[p] * Ts[p] * (2 if BF[p] else 4) for p in range(NPAIR)
    )
    FBUFS = int(os.environ.get("FBUFS", "6"))
    FBUFS = max(2, min(FBUFS, (150 * 1024) // max(1, fbytes)))

    with tile.TileContext(nc) as tc:
        with (
            tc.tile_pool(name="consts", bufs=1) as consts,
            tc.tile_pool(name="work", bufs=2) as work,
            tc.tile_pool(name="feats", bufs=FBUFS) as fpool,
            tc.tile_pool(name="soft", bufs=2) as soft,
            tc.tile_pool(name="ps", bufs=2, space="PSUM") as ps,
        ):
            # ---- input DMAs: trigger order = priority. SP queue carries the
            # critical keys/queries (HWDGE generation is a single serialized
            # ~630ns/DMA resource, so order matters); ACT queue takes the
            # weights + vl; DVE stays DMA-free for compute.
            # Full-K loads merge both batches into ONE DMA (one 625ns HWDGE
            # generation instead of two; the extra unread rows cost only
            # ~180ns of transfer when CK < 4).
            # merge pays (one 625ns gen saved vs extra rows transferred)
            # only when the pair already loads >= 3 chunks
            KMERGE = [kdma_merge and c >= 3 for c in CKs]
            KL = [K // 128 if m else c for c, m in zip(CKs, KMERGE)]
            keys_pair = []
            for p in range(NPAIR):
                kp = consts.tile([128, 2, KL[p], H], f32, name=f"keys_pair{p}")
                keys_pair.append(kp)

            def keys_dma(p):
                if KMERGE[p]:
                    nc.sync.dma_start(
                        out=keys_pair[p],
                        in_=keys_d[2 * p : 2 * p + 2].rearrange(
                            "b (c p) h -> p b c h", p=128
                        ),
                    )
                else:
                    for half in range(2):
                        nc.sync.dma_start(
                            out=keys_pair[p][:, half, :, :],
                            in_=keys_d[2 * p + half, 0 : CKs[p] * 128, :].rearrange(
                                "(c p) h -> p c h", p=128
                            ),
                        )

            keys_dma(0)
            # Wk gates the kT projection (the longest prep chain) -> first
            # on the ACT queue; Wq (query projection) next; vl last.
            Wk_sb = consts.tile([H, H], f32)
            nc.scalar.dma_start(out=Wk_sb, in_=Wk_d[:, :])
            Wq_sb = consts.tile([H, H], f32)
            nc.scalar.dma_start(out=Wq_sb, in_=Wq_d[:, :])
            q_all = consts.tile([Q, BPC, H], f32)
            nc.sync.dma_start(out=q_all, in_=queries_d.rearrange("b q h -> q b h"))
            vl_i = consts.tile([2, NPAIR, 1], i32)
            nc.scalar.dma_start(
                out=vl_i, in_=vlens_d.rearrange("(c r) o -> r c o", r=2)
            )

            keys_dma(1)

            # Sliding-window stationary matrix for the wv contraction:
            # wvmat[r, 64] = wv[r] for r < 64 ; wvmat[64+r, 128] = wv[r].
            # lhsT for q-index j is wvmat[:, 64-j : 192-j].
            wvmat0 = consts.tile([128, 256], f32r)
            nc.vector.memset(wvmat0.bitcast(f32), 0.0)
            nc.sync.dma_start(out=wvmat0[0:64, 64:65], in_=wv_d[:, :])
            nc.sync.dma_start(out=wvmat0[64:128, 128:129], in_=wv_d[:, :])

            # Row-selector for the additive-mask matmul.
            sel20 = consts.tile([2, 128], f32r)
            nc.vector.memset(sel20.bitcast(f32), 0.0)
            nc.vector.memset(sel20[0:1, 0:64].bitcast(f32), 1.0)
            # DVE can't write at partition base 1; stage row 1 and DMA it over.
            sel2row = consts.tile([1, 128], f32r)
            nc.vector.memset(sel2row.bitcast(f32), 0.0)
            nc.vector.memset(sel2row[0:1, 64:128].bitcast(f32), 1.0)
            nc.sync.dma_start(out=sel20[1:2, :], in_=sel2row)

            # ---- values: off the critical path, loaded per pair ----
            v0s = []
            for p in range(NPAIR):
                v0 = work.tile([128, 2, KL[p], V], f32, tag=f"v0_{p}", bufs=1)
                if KMERGE[p]:
                    nc.sync.dma_start(
                        out=v0,
                        in_=values_d[2 * p : 2 * p + 2].rearrange(
                            "b (c p) v -> p b c v", p=128
                        ),
                    )
                else:
                    for half in range(2):
                        nc.sync.dma_start(
                            out=v0[:, half, :, :],
                            in_=values_d[2 * p + half, 0 : CKs[p] * 128, :].rearrange(
                                "(c p) v -> p c v", p=128
                            ),
                        )
                v0s.append(v0)

            # ---- constants (DVE-finalized so PE waits merge) ----
            ident0 = consts.tile([128, 128], f32)
            make_identity(nc, ident0)
            ident = consts.tile([128, 128], f32)
            nc.vector.tensor_copy(ident, ident0)

            Wq2 = consts.tile([H, H], bf16)
            nc.vector.tensor_copy(Wq2, Wq_sb)
            Wk2 = consts.tile([H, H], bf16)
            nc.vector.tensor_copy(Wk2, Wk_sb)

            # small consts finalized on the (otherwise idle) Pool engine so
            # the DVE stream stays clear for the prep copies + first adds;
            # PE pays a single mergeable Pool-sem wait for them.
            iota2_i = consts.tile([2, 512], i32)
            nc.gpsimd.iota(iota2_i, pattern=[[1, 512]], base=0, channel_multiplier=0)
            iota2 = consts.tile([2, 512], f32)
            nc.gpsimd.tensor_copy(iota2, iota2_i)

            vl_f = consts.tile([2, NPAIR, 1], f32)
            nc.gpsimd.tensor_copy(vl_f, vl_i)

            wvmat = consts.tile([128, 256], f32r)
            nc.gpsimd.tensor_copy(wvmat, wvmat0)
            sel2 = consts.tile([2, 128], f32r)
            nc.gpsimd.tensor_copy(sel2, sel20)
            if any(BF):
                wvmat_bf = consts.tile([128, 256], bf16)
                nc.gpsimd.tensor_copy(wvmat_bf, wvmat0)
                sel2_bf = consts.tile([2, 128], bf16)
                nc.gpsimd.tensor_copy(sel2_bf, sel20)

            # Priming PE op: observe the DVE clock once so constant operands
            # never cost later matmuls a second wait slot.
            prime_ps = ps.tile([32, 32], f32, tag="tail", bufs=2)
            nc.tensor.transpose(prime_ps, ident[0:32, 0:32], ident[0:32, 0:32])
            # keep PE's busy-streak alive until the keys arrive so the first
            # projection matmuls run at the warm clock
            for _w in range(int(os.environ.get("WARMUP", "6"))):
                wps = ps.tile([128, 128], f32, tag="tail", bufs=2, name=f"wps{_w}")
                nc.tensor.transpose(wps, ident, ident)

            # ---- per-pair long-lived tiles ----
            kT_pair = []    # [128(h2), T(k)]
            qWT_pair = []   # [128(h2), 64(q)]
            amask = []      # [2, T]
            vals_pair = []  # [128, 2, CK, V]
            for p in range(NPAIR):
                fdt = bf16 if BF[p] else f32r
                kT_pair.append(consts.tile([128, Ts[p]], fdt, name=f"kT_pair{p}"))
                qWT_pair.append(consts.tile([128, Q], f32, name=f"qWT_pair{p}"))
                amask.append(
                    consts.tile([2, Ts[p]], bf16 if BF[p] else f32r, name=f"amask{p}")
                )
                vals_pair.append(
                    consts.tile([128, 2, CKs[p], V], bf16, name=f"vals{p}")
                )

            # ---- prep: ordered so the serial chain keys-DMA -> transpose ->
            # copy -> projection -> kT copy -> adds is as short as possible.
            # keysT half1's PSUM->SBUF copy runs on ACT (idle until the first
            # tanh) in parallel with DVE's half0 copy.
            def prep_pair(p):
                T, CK = Ts[p], CKs[p]
                keysT_pss = []
                for half in range(2):
                    keysT_ps = ps.tile(
                        [H, 512], f32, tag="prep", bufs=4,
                        name=f"keysT_ps{2*p+half}",
                    )
                    for c in range(CK):
                        nc.tensor.transpose(
                            keysT_ps[:, 128 * c : 128 * (c + 1)],
                            keys_pair[p][:, half, c, :],
                            ident,
                        )
                    keysT_pss.append(keysT_ps)
                qT_pss = []
                for half in range(2):
                    b = 2 * p + half
                    qT_ps = ps.tile([H, Q], f32, tag="prep", bufs=4, name=f"qT_ps{b}")
                    nc.tensor.transpose(qT_ps, q_all[:, b, :], ident[0:Q, 0:Q])
                    qT_pss.append(qT_ps)
                # PSUM->SBUF copies: for pair 0 BOTH keysT copies run on ACT
                # (idle until the first tanh) so DVE's FIFO is free for the
                # qT/qWT/kT copies that gate the first adds; pair 1's run on
                # DVE (mid-kernel, ACT is the bottleneck stream there)
                keysT_sb0 = work.tile([H, CK * 128], bf16, tag="keysT_sb", bufs=4)
                if p == 0 and act2:
                    nc.scalar.activation(
                        keysT_sb0, keysT_pss[0][:, 0 : CK * 128], Copy
                    )
                else:
                    nc.vector.tensor_copy(keysT_sb0, keysT_pss[0][:, 0 : CK * 128])
                qT_sbs = []
                for half in range(2):
                    qT_sb = work.tile([H, Q], bf16, tag="qT_sb", bufs=4)
                    nc.vector.tensor_copy(qT_sb, qT_pss[half])
                    qT_sbs.append(qT_sb)
                keysT_sb1 = work.tile([H, CK * 128], bf16, tag="keysT_sb", bufs=4)
                if p == 0:
                    # parallel with DVE's half-0 copy on the startup chain
                    nc.scalar.activation(
                        keysT_sb1, keysT_pss[1][:, 0 : CK * 128], Copy
                    )
                else:
                    # mid-kernel: ACT is the bottleneck stream, DVE has slack
                    nc.vector.tensor_copy(keysT_sb1, keysT_pss[1][:, 0 : CK * 128])
                keysT_sbs = [keysT_sb0, keysT_sb1]
                # projections, col-tiled so both halves land in one [128, *]
                # PSUM tile (half 1 via tile_position=(0, 64))
                qWT_ps = ps.tile([128, Q], f32, tag="prep", bufs=4, name=f"qWT_ps{p}")
                for half in range(2):
                    rows = slice(64 * half, 64 * half + 64)
                    nc.tensor.matmul(
                        qWT_ps[rows, :], lhsT=Wq2, rhs=qT_sbs[half],
                        start=True, stop=True,
                        tile_position=(0, 64 * half),
                    )
                kT_ps = ps.tile([128, 512], f32, tag="prep", bufs=4, name=f"kT_ps{p}")
                for half in range(2):
                    rows = slice(64 * half, 64 * half + 64)
                    nc.tensor.matmul(
                        kT_ps[rows, 0:T], lhsT=Wk2, rhs=keysT_sbs[half][:, 0:T],
                        start=True, stop=True,
                        tile_position=(0, 64 * half),
                    )
                if p == 0:
                    # ACT is idle before the first tanh; doing the qWT copy
                    # there lets DVE start the (bigger) kT copy immediately
                    nc.scalar.activation(qWT_pair[p], qWT_ps, Copy)
                else:
                    nc.vector.tensor_copy(qWT_pair[p], qWT_ps)
                if p == 0 and os.environ.get("KTACT", "0") == "1":
                    nc.scalar.activation(kT_pair[p], kT_ps[:, 0:T], Copy)
                else:
                    nc.vector.tensor_copy(kT_pair[p], kT_ps[:, 0:T])
                return kT_ps

            def late_mask(p):
                # mask row: needed only by the group-closing matmul (Pool)
                nc.gpsimd.tensor_scalar(
                    out=amask[p],
                    in0=iota2[:, 0 : Ts[p]],
                    scalar1=vl_f[:, p, :],
                    scalar2=NEG,
                    op0=mybir.AluOpType.is_ge,
                    op1=mybir.AluOpType.mult,
                )

            def late_vals(p):
                # values copy: needed only by the epilogue (Pool)
                src_v = (
                    v0s[p][:, :, 0 : CKs[p], :] if KL[p] > CKs[p] else v0s[p]
                )
                nc.gpsimd.tensor_copy(vals_pair[p], src_v)

            # ---- main: scores -> softmax -> output ----
            def main_pair(p, interleave=(), epilogues=(), kT_ps_early=None):
                T, CK = Ts[p], CKs[p]
                CHUNKS = CHUNKS_BY_PAIR[p]
                FMAX = FMAXs[p]
                fdt = bf16 if BF[p] else f32r
                wvm = wvmat_bf if BF[p] else wvmat
                sel = sel2_bf if BF[p] else sel2
                scores_ps = ps.tile([128, 512], f32, tag=f"scores{p}", bufs=1)
                j0 = 0
                ci = 0
                for csz in CHUNKS:
                    F = fpool.tile([128, FMAX, T], fdt, tag=f"F{p}", bufs=FBUFS)
                    # chunk 0 reads kT straight from PSUM (full-rate DVE, but
                    # ~0.6us earlier than waiting for the SBUF copy)
                    kT_src = (
                        kT_ps_early[:, 0:T] if (ci == 0 and kT_ps_early is not None)
                        else kT_pair[p]
                    )
                    for jj in range(csz):
                        j = j0 + jj
                        nc.vector.tensor_scalar_add(
                            out=F[:, jj, :],
                            in0=kT_src,
                            scalar1=qWT_pair[p][:, j : j + 1],
                        )
                    nc.scalar.activation(F[:, 0:csz, :], F[:, 0:csz, :], Tanh)
                    for jj in range(csz):
                        j = j0 + jj
                        nc.tensor.matmul(
                            scores_ps[:, 0:T],
                            lhsT=wvm[:, 64 - j : 192 - j],
                            rhs=F[:, jj, :],
                            start=(j == 0),
                            stop=False,
                        )
                    j0 += csz
                    ci += 1
                    for at, fn in interleave:
                        if ci == at:
                            fn()
                # additive mask closes the accumulation group: -1e9 where
                # k >= valid_len, so exp -> 0 there exactly
                nc.tensor.matmul(
                    scores_ps[:, 0:T], lhsT=sel, rhs=amask[p], start=False, stop=True
                )
                for fn in epilogues:
                    fn()

                def epilogue():
                    attn_sb = soft.tile([128, T], f32, tag=f"attn{p}", bufs=1)
                    sumexp = soft.tile([128, 1], f32, tag=f"sumexp{p}", bufs=1)
                    nc.scalar.activation(
                        attn_sb, scores_ps[:, 0:T], Exp, accum_out=sumexp
                    )
                    recip = soft.tile([128, 1], f32, tag=f"recip{p}", bufs=1)
                    nc.vector.reciprocal(recip, sumexp)

                    attnT_sb = soft.tile([128, CK, 128], bf16, tag=f"attnT{p}", bufs=1)
                    attnT_ps = ps.tile([128, 4, 128], f32, tag="tail", bufs=2)
                    for c in range(CK):
                        cw = min(128, T - 128 * c)
                        nc.tensor.transpose(
                            attnT_ps[0:cw, c, :],
                            attn_sb[:, 128 * c : 128 * c + cw],
                            ident,
                        )
                    for c in range(CK):
                        cw = min(128, T - 128 * c)
                        nc.vector.tensor_copy(
                            attnT_sb[0:cw, c, :], attnT_ps[0:cw, c, :]
                        )

                    out_pair = soft.tile([128, V], f32, tag=f"out_pair{p}", bufs=1)
                    out_ps = ps.tile([128, V], f32, tag="tail", bufs=2)
                    for half in range(2):
                        rows = slice(64 * half, 64 * half + 64)
                        for c in range(CK):
                            cw = min(128, T - 128 * c)
                            nc.tensor.matmul(
                                out_ps[rows, :],
                                lhsT=attnT_sb[0:cw, c, rows],
                                rhs=vals_pair[p][0:cw, half, c, :],
                                start=(c == 0),
                                stop=(c == CK - 1),
                                tile_position=(0, 64 * half),
                            )
                    nc.vector.tensor_scalar_mul(out=out_pair, in0=out_ps, scalar1=recip)
                    nc.sync.dma_start(
                        out=out_d[2 * p : 2 * p + 2].rearrange("b q v -> (b q) v"),
                        in_=out_pair,
                    )

                return epilogue

            EPI_AFTER = int(os.environ.get("EPI_AFTER", "2"))
            VALS_AFTER = int(os.environ.get("VALS_AFTER", "5"))
            PREP1_AFTER = int(os.environ.get("PREP1_AFTER", "2"))
            PREP1_MS = prep1_ms

            def prep_pair1():
                # optionally push pair 1's prep later in the scheduler's
                # virtual timeline so it can't displace pair 0's first adds
                # in the in-order DVE stream
                if PREP1_MS > 0:
                    with tc.tile_wait_until(PREP1_MS):
                        prep_pair(1)
                else:
                    prep_pair(1)

            kT_ps0 = prep_pair(0)
            # pair 1's prep is issued a couple of chunks into main 0 so the
            # scheduler can't hoist its transposes/copies into pair 0's
            # serial prep -> first-tanh chain.
            epi0 = main_pair(
                0,
                interleave=[
                    (1, lambda: late_mask(0)),
                    (PREP1_AFTER, prep_pair1),
                    (VALS_AFTER, lambda: late_vals(0)),
                ],
                kT_ps_early=kT_ps0 if os.environ.get("KTPS", "0") == "1" else None,
            )
            epi1 = main_pair(
                1,
                interleave=[
                    (1, lambda: late_mask(1)),
                    (2, lambda: late_vals(1)),
                    (EPI_AFTER, epi0),
                ],
            )
            epi1()

    nc.compile()
    return nc


def _compositions(n, m):
    """All ways to write n as ordered sum of m positive ints."""
    if m == 1:
        yield (n,)
        return
    for first in range(1, n - m + 2):
        for rest in _compositions(n - first, m - 1):
            yield (first,) + rest


_TIME_CACHE = {}


def _class_time(A, Bx):
    """Modeled per-core exec time of the (A, B) program via TimelineSim,
    minimized over a small PREP1_MS sweep (the virtual-time slot for pair
    1's prep interacts with the class's work size). Returns ns; caches
    (ns, best_prep1_ms). Falls back to a load heuristic without the sim."""
    key = (A, Bx)
    if key not in _TIME_CACHE:
        try:
            from concourse.timeline_sim import TimelineSim

            best = None
            for ms in (0.012, 0.02):
                for c1 in (
                    None, (22, 22, 14, 4, 2), (18, 18, 16, 6, 4, 2)
                ):
                    for km in (True, False):
                        nc = build_nc(A, Bx, prep1_ms=ms, ch1=c1, kdma_merge=km)
                        t = float(TimelineSim(nc, trace=False).simulate())
                        if best is None or t < best[0]:
                            best = (t, ms, c1, km)
            _TIME_CACHE[key] = best
        except Exception:
            _TIME_CACHE[key] = (53.3 * (A + Bx) + 10.0 * Bx, None, None, True)
    return _TIME_CACHE[key][0]


def best_cfg(A, Bx):
    _class_time(A, Bx)
    return _TIME_CACHE[(A, Bx)][1:]


def best_prep1_ms(A, Bx):
    return best_cfg(A, Bx)[0]


def plan(vl):
    """Partition 32 batches into 8 cores x (pair0, pair1) and <= MAXPROGS
    program classes. Returns list of classes:
      {"T0", "T1", "cores": [list of 4 original batch indices per core]}
    Cores across classes are disjoint and cover all batches. Candidate
    partitions are scored by the max TimelineSim-modeled class time.
    """
    vl = np.asarray(vl).reshape(-1).astype(np.int64)
    assert vl.shape[0] == B
    order = np.argsort(-vl, kind="stable")
    pairs = [(int(order[2 * i]), int(order[2 * i + 1])) for i in range(B // 2)]
    # pair extent = max valid_len, rounded up to 8 (fp32r ISA granularity)
    ext = [min(K, (int(vl[p[0]]) + 7) // 8 * 8) for p in pairs]

    npair = len(pairs)          # 16
    ncore = npair // 2          # 8
    slot0 = list(range(ncore))            # pair indices 0..7  (big)
    slot1 = list(range(ncore, npair))     # pair indices 8..15 (small)

    # enumerate candidate partitions; prefilter by load to bound the number
    # of distinct (A, B) programs that get TimelineSim-scored
    cands = []
    for m in range(1, min(MAXPROGS, ncore) + 1):
        for comp in _compositions(ncore, m):
            # class k slot0 group: contiguous slice of slot0 pairs (desc);
            # class k slot1 group: contiguous slice of slot1 pairs, assigned
            # in REVERSE class order so the largest-extent class gets the
            # smallest slot1 extents.
            s0_groups = []
            off = 0
            for nk in comp:
                s0_groups.append(slot0[off : off + nk])
                off += nk
            s1_groups = [None] * m
            off = 0
            for k in reversed(range(m)):
                nk = comp[k]
                s1_groups[k] = slot1[off : off + nk]
                off += nk
            classes = []
            loads = []
            for k in range(m):
                A = max(ext[i] for i in s0_groups[k])
                Bx = max(ext[i] for i in s1_groups[k])
                A, Bx = max(A, Bx), min(A, Bx)
                loads.append(A + Bx)
                classes.append((A, Bx, s0_groups[k], s1_groups[k]))
            cands.append((max(loads), sum(loads), m, classes))
    cands.sort(key=lambda c: c[:3])
    best_load = cands[0][0]
    # keep partitions within 48 of the best max-load; sim-score those
    cands = [c for c in cands if c[0] <= best_load + 96][:40]
    best = None
    for _, _, m, classes in cands:
        t = max(_class_time(A, Bx) for A, Bx, _, _ in classes)
        score = (t, sum(_class_time(A, Bx) for A, Bx, _, _ in classes), m)
        if best is None or score < best[0]:
            best = (score, classes)

    _, classes = best
    out = []
    for A, Bx, a_pairs, b_pairs in classes:
        cores = []
        for i in range(len(a_pairs)):
            pa = pairs[a_pairs[i]]
            pb = pairs[b_pairs[len(b_pairs) - 1 - i]]
            cores.append([pa[0], pa[1], pb[0], pb[1]])
        out.append({"T0": int(A), "T1": int(Bx), "cores": cores})
    return out


def get_nc(T0, T1):
    ms, c1, km = best_cfg(T0, T1)
    key = (T0, T1, ms, c1, km, os.environ.get("MM_RELAXED", "1"),
           os.environ.get("CH0", ""), os.environ.get("CH1", ""),
           os.environ.get("FBUFS", ""), os.environ.get("EPI_AFTER", ""))
    if key not in _NC_CACHE:
        _NC_CACHE[key] = build_nc(T0, T1, prep1_ms=ms, ch1=c1, kdma_merge=km)
    return _NC_CACHE[key]


def kernel(queries, keys, values, valid_lens, Wq, Wk, wv):
    from concourse.bass_utils import run_bass_kernel_spmd

    queries = np.ascontiguousarray(queries, dtype=np.float32)
    keys = np.ascontiguousarray(keys, dtype=np.float32)
    values = np.ascontiguousarray(values, dtype=np.float32)
    vl = np.ascontiguousarray(np.asarray(valid_lens).reshape(B), dtype=np.int32)
    Wq = np.ascontiguousarray(Wq, dtype=np.float32)
    Wk = np.ascontiguousarray(Wk, dtype=np.float32)
    wv2 = np.ascontiguousarray(wv, dtype=np.float32).reshape(H, 1)

    out = np.empty((B, Q, V), dtype=np.float32)
    for cls in plan(vl):
        nc = get_nc(cls["T0"], cls["T1"])
        in_maps = []
        for bidx in cls["cores"]:
            in_maps.append(
                {
                    "queries": queries[bidx],
                    "keys": keys[bidx],
                    "values": values[bidx],
                    "valid_lens": vl[bidx].reshape(BPC, 1),
                    "Wq": Wq,
                    "Wk": Wk,
                    "wv": wv2,
                }
            )
        res = run_bass_kernel_spmd(nc, in_maps, core_ids=list(range(len(in_maps))))
        for bidx, r in zip(cls["cores"], res.results):
            out[bidx] = r["out"]
    return out


if __name__ == "__main__":
    rng = np.random.default_rng(0)
    q = rng.standard_normal((B, Q, H), dtype=np.float32)
    k = rng.standard_normal((B, K, H), dtype=np.float32)
    v = rng.standard_normal((B, K, V), dtype=np.float32)
    vl = rng.integers(1, K + 1, size=(B,)).astype(np.int32)
    Wq = rng.standard_normal((H, H), dtype=np.float32) / np.sqrt(H)
    Wk = rng.standard_normal((H, H), dtype=np.float32) / np.sqrt(H)
    wv = rng.standard_normal((H,), dtype=np.float32) / np.sqrt(H)
    out = kernel(queries=q, keys=k, values=v, valid_lens=vl, Wq=Wq, Wk=Wk, wv=wv)
    print(out.shape, out.dtype, np.abs(out).mean())


# revision 70
# speedup vs baseline: 1.0004x; 1.0004x over previous
"""Additive (Bahdanau) attention on 8 Trainium2 NeuronCores.

Full inputs in, full output out. Data-parallel over batch with
input-value-specialized programs: the masked softmax makes key positions
k >= valid_len irrelevant (they get -1e9 before exp), so each compiled
program only computes feats/tanh up to a per-pair key extent T (rounded
to 8 -- fp32r matmuls need an even moving free count on real HW, and
f32r with a partial-partition destination is rejected outright, hence
plain f32 for the projection / output matmuls). Batches are sorted by
valid_len, paired (2 batches pack the 128 SBUF partitions: rows 0-63 =
batch even's 64 hidden dims, 64-127 = batch odd), and pairs are
assigned to cores big-with-small. Cores are grouped into <= MAXPROGS
classes; each class compiles one program parameterized by (T0, T1) =
slot extents (T0 >= T1 so the serial epilogue hangs off the small
pair). Candidate partitions are scored by TimelineSim (max modeled
class time, minimized also over the PREP1_MS virtual-schedule knob).

The feats pipeline runs in bf16 by default (BFMIN=600): tanh cost on
ACT is dtype-independent, but DVE adds get the 4x perf mode and PE
matmuls run 1 cyc/row at any width (f32r only reaches that at moving
width >= 256). Output error ~2e-3 absmax-relative vs the 2e-2 gate.

Per (batch b, query j):  scores[j, k] = sum_h wv[h] * tanh(qW[b,j,h] + kW[b,k,h])
Layout on chip: h on partitions, k on free dim.
  - DVE tensor_scalar_add broadcasts qW[:, j] onto kW (2x fp32 / 4x bf16)
  - ACT does one big in-place tanh over a j-chunk at a time (the
    bottleneck engine: ~0.83ns/elem/lane, no fast dtype mode)
  - PE contracts with wv via a sliding-window stationary matrix,
    accumulating all 64 j's into one [128, T] PSUM bank; the additive
    -1e9 mask matmul CLOSES the group (start comes from j=0) so the
    valid_lens DMA stays off the startup critical path
  - ACT Exp (PSUM->SBUF) with fused accum_out row-sum, DVE reciprocal
  - PE transposes attn, accumulated matmul vs values, scale rows by
    1/sum, DMA out. Pair 0's exp/epilogue is interleaved into pair 1's
    main loop so ACT never stalls on PE's matmul tail.

Scheduling constraint: matmul/transpose instructions lower to a fused
LDWEIGHTS (S3_LW) slot that fits only ONE semaphore wait. So every PE
operand's last writer is kept on DVE or ACT (per-engine sems merge into
one monotonic wait); DMA-landed constants get a small DVE copy first,
and one priming PE op observes the DVE clock early.
"""

import os
import sys

import numpy as np

for _p in ("/opt/trn_rl_repo", "/root/.axon_site/_ro/trn_rl_repo"):
    if os.path.isdir(_p) and _p not in sys.path:
        sys.path.append(_p)

B, Q, K, H, V = 32, 64, 512, 64, 64
NCORES = 8
BPC = B // NCORES          # batches per core
NPAIR = BPC // 2           # batch pairs per core
NEG = -1e9
MAXPROGS = int(os.environ.get("MAXPROGS", "8"))

_NC_CACHE = {}


def _chunks(total, role):
    """j-chunk schedule summing to `total` (=Q).

    Head taper: chunk i+1's DVE adds must finish inside chunk i's tanh
    plus the accumulated slack, or ACT stalls. With adds ~0.8x tanh per j
    the sizes can grow ~1.25x per step from a small start.
    """
    env = os.environ.get("CH0" if role == "head" else "CH1")
    if env:
        c = [int(x) for x in env.split(",")]
    elif role == "head":
        c = [2, 4, 8, 16, 17, 17]
    else:  # tail: big first, tiny last so the epilogue starts asap
        c = [16, 16, 12, 8, 6, 4, 2]
    assert sum(c) == total, c
    return c


def build_nc(T0=512, T1=512, mm_relaxed=None, prep1_ms=None, act2=None,
             ch1=None, ch0=None, kdma_merge=True, mask_mid=True):
    """One-core program for pair extents (T0, T1), T0 >= T1."""
    if mm_relaxed is None:
        mm_relaxed = os.environ.get("MM_RELAXED", "1") == "1"
    if prep1_ms is None:
        prep1_ms = float(os.environ.get("PREP1_MS", "0.02"))
    if act2 is None:
        act2 = os.environ.get("ACT2", "0") == "1"
    import concourse.bass as bass  # noqa: F401
    from concourse import mybir
    from concourse import tile
    from concourse.masks import make_identity

    f32 = mybir.dt.float32
    f32r = mybir.dt.float32r if mm_relaxed else mybir.dt.float32
    bf16 = mybir.dt.bfloat16
    i32 = mybir.dt.int32
    Tanh = mybir.ActivationFunctionType.Tanh
    Exp = mybir.ActivationFunctionType.Exp
    Copy = mybir.ActivationFunctionType.Copy

    from concourse import bacc

    nc = bacc.Bacc("TRN2", target_bir_lowering=False, debug=False, num_devices=NCORES)

    queries_d = nc.dram_tensor("queries", [BPC, Q, H], f32, kind="ExternalInput")
    keys_d = nc.dram_tensor("keys", [BPC, K, H], f32, kind="ExternalInput")
    values_d = nc.dram_tensor("values", [BPC, K, V], f32, kind="ExternalInput")
    vlens_d = nc.dram_tensor("valid_lens", [BPC, 1], i32, kind="ExternalInput")
    Wq_d = nc.dram_tensor("Wq", [H, H], f32, kind="ExternalInput")
    Wk_d = nc.dram_tensor("Wk", [H, H], f32, kind="ExternalInput")
    wv_d = nc.dram_tensor("wv", [H, 1], f32r, kind="ExternalInput")
    out_d = nc.dram_tensor("out", [BPC, Q, V], f32, kind="ExternalOutput")

    # fp32r matmuls require an even moving free count (and 8B-aligned
    # patterns); round extents to multiples of 8.
    Ts = [min(K, (int(t) + 7) // 8 * 8) for t in (T0, T1)]
    assert Ts[0] >= Ts[1] >= 8 and Ts[0] <= K
    # bf16 feats pipeline: required for T < 256 (f32r matmuls are 4x slower
    # below 256 moving width); optional above it (BFMIN lowers the cutoff)
    BFMIN = int(os.environ.get("BFMIN", "600"))
    BF = [t < BFMIN or t < 256 for t in Ts]
    CKs = [(t + 127) // 128 for t in Ts]    # 128-wide key chunks loaded/used
    CHUNKS_BY_PAIR = [
        list(ch0) if ch0 is not None else _chunks(Q, "head"),
        list(ch1) if ch1 is not None else _chunks(Q, "tail"),
    ]
    FMAXs = [max(c) for c in CHUNKS_BY_PAIR]
    # F pools must fit SBUF (192KB/partition) alongside ~40KB of other
    # tiles; shrink the ring when the extents are large.
    fbytes = sum(
        FMAXs[p] * Ts[p] * (2 if BF[p] else 4) for p in range(NPAIR)
    )
    FBUFS = int(os.environ.get("FBUFS", "6"))
    FBUFS = max(2, min(FBUFS, (150 * 1024) // max(1, fbytes)))

    with tile.TileContext(nc) as tc:
        with (
            tc.tile_pool(name="consts", bufs=1) as consts,
            tc.tile_pool(name="work", bufs=2) as work,
            tc.tile_pool(name="feats", bufs=FBUFS) as fpool,
            tc.tile_pool(name="soft", bufs=2) as soft,
            tc.tile_pool(name="ps", bufs=2, space="PSUM") as ps,
        ):
            # ---- input DMAs: trigger order = priority. SP queue carries the
            # critical keys/queries (HWDGE generation is a single serialized
            # ~630ns/DMA resource, so order matters); ACT queue takes the
            # weights + vl; DVE stays DMA-free for compute.
            # Full-K loads merge both batches into ONE DMA (one 625ns HWDGE
            # generation instead of two; the extra unread rows cost only
            # ~180ns of transfer when CK < 4).
            # merge pays (one 625ns gen saved vs extra rows transferred)
            # only when the pair already loads >= 3 chunks
            KMERGE = [kdma_merge and c >= 3 for c in CKs]
            KL = [K // 128 if m else c for c, m in zip(CKs, KMERGE)]
            keys_pair = []
            for p in range(NPAIR):
                kp = consts.tile([128, 2, KL[p], H], f32, name=f"keys_pair{p}")
                keys_pair.append(kp)

            def keys_dma(p):
                if KMERGE[p]:
                    nc.sync.dma_start(
                        out=keys_pair[p],
                        in_=keys_d[2 * p : 2 * p + 2].rearrange(
                            "b (c p) h -> p b c h", p=128
                        ),
                    )
                else:
                    for half in range(2):
                        nc.sync.dma_start(
                            out=keys_pair[p][:, half, :, :],
                            in_=keys_d[2 * p + half, 0 : CKs[p] * 128, :].rearrange(
                                "(c p) h -> p c h", p=128
                            ),
                        )

            keys_dma(0)
            # Wk gates the kT projection (the longest prep chain) -> first
            # on the ACT queue; Wq (query projection) next; vl last.
            Wk_sb = consts.tile([H, H], f32)
            nc.scalar.dma_start(out=Wk_sb, in_=Wk_d[:, :])
            Wq_sb = consts.tile([H, H], f32)
            nc.scalar.dma_start(out=Wq_sb, in_=Wq_d[:, :])
            q_all = consts.tile([Q, BPC, H], f32)
            nc.sync.dma_start(out=q_all, in_=queries_d.rearrange("b q h -> q b h"))
            vl_i = consts.tile([2, NPAIR, 1], i32)
            nc.scalar.dma_start(
                out=vl_i, in_=vlens_d.rearrange("(c r) o -> r c o", r=2)
            )

            keys_dma(1)

            # Sliding-window stationary matrix for the wv contraction:
            # wvmat[r, 64] = wv[r] for r < 64 ; wvmat[64+r, 128] = wv[r].
            # lhsT for q-index j is wvmat[:, 64-j : 192-j].
            wvmat0 = consts.tile([128, 256], f32r)
            nc.vector.memset(wvmat0.bitcast(f32), 0.0)
            nc.sync.dma_start(out=wvmat0[0:64, 64:65], in_=wv_d[:, :])
            nc.sync.dma_start(out=wvmat0[64:128, 128:129], in_=wv_d[:, :])

            # Row-selector for the additive-mask matmul.
            sel20 = consts.tile([2, 128], f32r)
            nc.vector.memset(sel20.bitcast(f32), 0.0)
            nc.vector.memset(sel20[0:1, 0:64].bitcast(f32), 1.0)
            # DVE can't write at partition base 1; stage row 1 and DMA it over.
            sel2row = consts.tile([1, 128], f32r)
            nc.vector.memset(sel2row.bitcast(f32), 0.0)
            nc.vector.memset(sel2row[0:1, 64:128].bitcast(f32), 1.0)
            nc.sync.dma_start(out=sel20[1:2, :], in_=sel2row)

            # ---- values: off the critical path, loaded per pair ----
            v0s = []
            for p in range(NPAIR):
                v0 = work.tile([128, 2, KL[p], V], f32, tag=f"v0_{p}", bufs=1)
                if KMERGE[p]:
                    nc.sync.dma_start(
                        out=v0,
                        in_=values_d[2 * p : 2 * p + 2].rearrange(
                            "b (c p) v -> p b c v", p=128
                        ),
                    )
                else:
                    for half in range(2):
                        nc.sync.dma_start(
                            out=v0[:, half, :, :],
                            in_=values_d[2 * p + half, 0 : CKs[p] * 128, :].rearrange(
                                "(c p) v -> p c v", p=128
                            ),
                        )
                v0s.append(v0)

            # ---- constants (DVE-finalized so PE waits merge) ----
            ident0 = consts.tile([128, 128], f32)
            make_identity(nc, ident0)
            ident = consts.tile([128, 128], f32)
            nc.vector.tensor_copy(ident, ident0)

            Wq2 = consts.tile([H, H], bf16)
            nc.vector.tensor_copy(Wq2, Wq_sb)
            Wk2 = consts.tile([H, H], bf16)
            nc.vector.tensor_copy(Wk2, Wk_sb)

            # small consts finalized on the (otherwise idle) Pool engine so
            # the DVE stream stays clear for the prep copies + first adds;
            # PE pays a single mergeable Pool-sem wait for them.
            iota2_i = consts.tile([2, 512], i32)
            nc.gpsimd.iota(iota2_i, pattern=[[1, 512]], base=0, channel_multiplier=0)
            iota2 = consts.tile([2, 512], f32)
            nc.gpsimd.tensor_copy(iota2, iota2_i)

            vl_f = consts.tile([2, NPAIR, 1], f32)
            nc.gpsimd.tensor_copy(vl_f, vl_i)

            wvmat = consts.tile([128, 256], f32r)
            nc.gpsimd.tensor_copy(wvmat, wvmat0)
            sel2 = consts.tile([2, 128], f32r)
            nc.gpsimd.tensor_copy(sel2, sel20)
            if any(BF):
                wvmat_bf = consts.tile([128, 256], bf16)
                nc.gpsimd.tensor_copy(wvmat_bf, wvmat0)
                sel2_bf = consts.tile([2, 128], bf16)
                nc.gpsimd.tensor_copy(sel2_bf, sel20)

            # Priming PE op: observe the DVE clock once so constant operands
            # never cost later matmuls a second wait slot.
            prime_ps = ps.tile([32, 32], f32, tag="tail", bufs=2)
            nc.tensor.transpose(prime_ps, ident[0:32, 0:32], ident[0:32, 0:32])
            # keep PE's busy-streak alive until the keys arrive so the first
            # projection matmuls run at the warm clock
            for _w in range(int(os.environ.get("WARMUP", "6"))):
                wps = ps.tile([128, 128], f32, tag="tail", bufs=2, name=f"wps{_w}")
                nc.tensor.transpose(wps, ident, ident)

            # ---- per-pair long-lived tiles ----
            kT_pair = []    # [128(h2), T(k)]
            qWT_pair = []   # [128(h2), 64(q)]
            amask = []      # [2, T]
            vals_pair = []  # [128, 2, CK, V]
            for p in range(NPAIR):
                fdt = bf16 if BF[p] else f32r
                kT_pair.append(consts.tile([128, Ts[p]], fdt, name=f"kT_pair{p}"))
                qWT_pair.append(consts.tile([128, Q], f32, name=f"qWT_pair{p}"))
                amask.append(
                    consts.tile([2, Ts[p]], bf16 if BF[p] else f32r, name=f"amask{p}")
                )
                vals_pair.append(
                    consts.tile([128, 2, CKs[p], V], bf16, name=f"vals{p}")
                )

            # ---- prep: ordered so the serial chain keys-DMA -> transpose ->
            # copy -> projection -> kT copy -> adds is as short as possible.
            # keysT half1's PSUM->SBUF copy runs on ACT (idle until the first
            # tanh) in parallel with DVE's half0 copy.
            def prep_pair(p):
                T, CK = Ts[p], CKs[p]
                keysT_pss = []
                for half in range(2):
                    keysT_ps = ps.tile(
                        [H, 512], f32, tag="prep", bufs=4,
                        name=f"keysT_ps{2*p+half}",
                    )
                    for c in range(CK):
                        nc.tensor.transpose(
                            keysT_ps[:, 128 * c : 128 * (c + 1)],
                            keys_pair[p][:, half, c, :],
                            ident,
                        )
                    keysT_pss.append(keysT_ps)
                qT_pss = []
                for half in range(2):
                    b = 2 * p + half
                    qT_ps = ps.tile([H, Q], f32, tag="prep", bufs=4, name=f"qT_ps{b}")
                    nc.tensor.transpose(qT_ps, q_all[:, b, :], ident[0:Q, 0:Q])
                    qT_pss.append(qT_ps)
                # PSUM->SBUF copies: for pair 0 BOTH keysT copies run on ACT
                # (idle until the first tanh) so DVE's FIFO is free for the
                # qT/qWT/kT copies that gate the first adds; pair 1's run on
                # DVE (mid-kernel, ACT is the bottleneck stream there)
                keysT_sb0 = work.tile([H, CK * 128], bf16, tag="keysT_sb", bufs=4)
                if p == 0 and act2:
                    nc.scalar.activation(
                        keysT_sb0, keysT_pss[0][:, 0 : CK * 128], Copy
                    )
                else:
                    nc.vector.tensor_copy(keysT_sb0, keysT_pss[0][:, 0 : CK * 128])
                qT_sbs = []
                for half in range(2):
                    qT_sb = work.tile([H, Q], bf16, tag="qT_sb", bufs=4)
                    nc.vector.tensor_copy(qT_sb, qT_pss[half])
                    qT_sbs.append(qT_sb)
                keysT_sb1 = work.tile([H, CK * 128], bf16, tag="keysT_sb", bufs=4)
                if p == 0:
                    # parallel with DVE's half-0 copy on the startup chain
                    nc.scalar.activation(
                        keysT_sb1, keysT_pss[1][:, 0 : CK * 128], Copy
                    )
                else:
                    # mid-kernel: ACT is the bottleneck stream, DVE has slack
                    nc.vector.tensor_copy(keysT_sb1, keysT_pss[1][:, 0 : CK * 128])
                keysT_sbs = [keysT_sb0, keysT_sb1]
                # projections, col-tiled so both halves land in one [128, *]
                # PSUM tile (half 1 via tile_position=(0, 64))
                qWT_ps = ps.tile([128, Q], f32, tag="prep", bufs=4, name=f"qWT_ps{p}")
                for half in range(2):
                    rows = slice(64 * half, 64 * half + 64)
                    nc.tensor.matmul(
                        qWT_ps[rows, :], lhsT=Wq2, rhs=qT_sbs[half],
                        start=True, stop=True,
                        tile_position=(0, 64 * half),
                    )
                kT_ps = ps.tile([128, 512], f32, tag="prep", bufs=4, name=f"kT_ps{p}")
                for half in range(2):
                    rows = slice(64 * half, 64 * half + 64)
                    nc.tensor.matmul(
                        kT_ps[rows, 0:T], lhsT=Wk2, rhs=keysT_sbs[half][:, 0:T],
                        start=True, stop=True,
                        tile_position=(0, 64 * half),
                    )
                if p == 0:
                    # ACT is idle before the first tanh; doing the qWT copy
                    # there lets DVE start the (bigger) kT copy immediately
                    nc.scalar.activation(qWT_pair[p], qWT_ps, Copy)
                else:
                    nc.vector.tensor_copy(qWT_pair[p], qWT_ps)
                if p == 0 and os.environ.get("KTACT", "0") == "1":
                    nc.scalar.activation(kT_pair[p], kT_ps[:, 0:T], Copy)
                else:
                    nc.vector.tensor_copy(kT_pair[p], kT_ps[:, 0:T])
                return kT_ps

            def late_mask(p):
                # mask row: needed only by the group-closing matmul (Pool)
                nc.gpsimd.tensor_scalar(
                    out=amask[p],
                    in0=iota2[:, 0 : Ts[p]],
                    scalar1=vl_f[:, p, :],
                    scalar2=NEG,
                    op0=mybir.AluOpType.is_ge,
                    op1=mybir.AluOpType.mult,
                )

            def late_vals(p):
                # values copy: needed only by the epilogue (Pool)
                src_v = (
                    v0s[p][:, :, 0 : CKs[p], :] if KL[p] > CKs[p] else v0s[p]
                )
                nc.gpsimd.tensor_copy(vals_pair[p], src_v)

            # ---- main: scores -> softmax -> output ----
            def main_pair(p, interleave=(), epilogues=(), kT_ps_early=None):
                T, CK = Ts[p], CKs[p]
                CHUNKS = CHUNKS_BY_PAIR[p]
                FMAX = FMAXs[p]
                fdt = bf16 if BF[p] else f32r
                wvm = wvmat_bf if BF[p] else wvmat
                sel = sel2_bf if BF[p] else sel2
                scores_ps = ps.tile([128, 512], f32, tag=f"scores{p}", bufs=1)
                j0 = 0
                ci = 0
                for csz in CHUNKS:
                    F = fpool.tile([128, FMAX, T], fdt, tag=f"F{p}", bufs=FBUFS)
                    # chunk 0 reads kT straight from PSUM (full-rate DVE, but
                    # ~0.6us earlier than waiting for the SBUF copy)
                    kT_src = (
                        kT_ps_early[:, 0:T] if (ci == 0 and kT_ps_early is not None)
                        else kT_pair[p]
                    )
                    for jj in range(csz):
                        j = j0 + jj
                        nc.vector.tensor_scalar_add(
                            out=F[:, jj, :],
                            in0=kT_src,
                            scalar1=qWT_pair[p][:, j : j + 1],
                        )
                    nc.scalar.activation(F[:, 0:csz, :], F[:, 0:csz, :], Tanh)
                    for jj in range(csz):
                        j = j0 + jj
                        nc.tensor.matmul(
                            scores_ps[:, 0:T],
                            lhsT=wvm[:, 64 - j : 192 - j],
                            rhs=F[:, jj, :],
                            start=(j == 0),
                            stop=(mask_mid and j == Q - 1),
                        )
                    j0 += csz
                    ci += 1
                    if mask_mid and ci == MASK_AFTER:
                        # additive mask joins MID-group (j=0 already opened
                        # it): -1e9 where k >= valid_len, so exp -> 0 there
                        # exactly. Off the serial close chain: after the
                        # final tanh only the last j-matmuls remain.
                        nc.tensor.matmul(
                            scores_ps[:, 0:T], lhsT=sel, rhs=amask[p],
                            start=False, stop=False,
                        )
                    for at, fn in interleave:
                        if ci == at:
                            fn()
                if not mask_mid:
                    # additive mask closes the accumulation group
                    nc.tensor.matmul(
                        scores_ps[:, 0:T], lhsT=sel, rhs=amask[p],
                        start=False, stop=True,
                    )
                for fn in epilogues:
                    fn()

                def epilogue():
                    attn_sb = soft.tile([128, T], f32, tag=f"attn{p}", bufs=1)
                    sumexp = soft.tile([128, 1], f32, tag=f"sumexp{p}", bufs=1)
                    nc.scalar.activation(
                        attn_sb, scores_ps[:, 0:T], Exp, accum_out=sumexp
                    )
                    recip = soft.tile([128, 1], f32, tag=f"recip{p}", bufs=1)
                    nc.vector.reciprocal(recip, sumexp)

                    attnT_sb = soft.tile([128, CK, 128], bf16, tag=f"attnT{p}", bufs=1)
                    attnT_ps = ps.tile([128, 4, 128], f32, tag="tail", bufs=2)
                    for c in range(CK):
                        cw = min(128, T - 128 * c)
                        nc.tensor.transpose(
                            attnT_ps[0:cw, c, :],
                            attn_sb[:, 128 * c : 128 * c + cw],
                            ident,
                        )
                    for c in range(CK):
                        cw = min(128, T - 128 * c)
                        nc.vector.tensor_copy(
                            attnT_sb[0:cw, c, :], attnT_ps[0:cw, c, :]
                        )

                    out_pair = soft.tile([128, V], f32, tag=f"out_pair{p}", bufs=1)
                    out_ps = ps.tile([128, V], f32, tag="tail", bufs=2)
                    for half in range(2):
                        rows = slice(64 * half, 64 * half + 64)
                        for c in range(CK):
                            cw = min(128, T - 128 * c)
                            nc.tensor.matmul(
                                out_ps[rows, :],
                                lhsT=attnT_sb[0:cw, c, rows],
                                rhs=vals_pair[p][0:cw, half, c, :],
                                start=(c == 0),
                                stop=(c == CK - 1),
                                tile_position=(0, 64 * half),
                            )
                    nc.vector.tensor_scalar_mul(out=out_pair, in0=out_ps, scalar1=recip)
                    nc.sync.dma_start(
                        out=out_d[2 * p : 2 * p + 2].rearrange("b q v -> (b q) v"),
                        in_=out_pair,
                    )

                return epilogue

            EPI_AFTER = int(os.environ.get("EPI_AFTER", "2"))
            MASK_AFTER = int(os.environ.get("MASK_AFTER", "2"))
            VALS_AFTER = int(os.environ.get("VALS_AFTER", "5"))
            PREP1_AFTER = int(os.environ.get("PREP1_AFTER", "2"))
            PREP1_MS = prep1_ms

            def prep_pair1():
                # optionally push pair 1's prep later in the scheduler's
                # virtual timeline so it can't displace pair 0's first adds
                # in the in-order DVE stream
                if PREP1_MS > 0:
                    with tc.tile_wait_until(PREP1_MS):
                        prep_pair(1)
                else:
                    prep_pair(1)

            kT_ps0 = prep_pair(0)
            # pair 1's prep is issued a couple of chunks into main 0 so the
            # scheduler can't hoist its transposes/copies into pair 0's
            # serial prep -> first-tanh chain.
            epi0 = main_pair(
                0,
                interleave=[
                    (1, lambda: late_mask(0)),
                    (PREP1_AFTER, prep_pair1),
                    (VALS_AFTER, lambda: late_vals(0)),
                ],
                kT_ps_early=kT_ps0 if os.environ.get("KTPS", "0") == "1" else None,
            )
            epi1 = main_pair(
                1,
                interleave=[
                    (1, lambda: late_mask(1)),
                    (2, lambda: late_vals(1)),
                    (EPI_AFTER, epi0),
                ],
            )
            epi1()

    nc.compile()
    return nc


def _compositions(n, m):
    """All ways to write n as ordered sum of m positive ints."""
    if m == 1:
        yield (n,)
        return
    for first in range(1, n - m + 2):
        for rest in _compositions(n - first, m - 1):
            yield (first,) + rest


_TIME_CACHE = {}


def _class_time(A, Bx):
    """Modeled per-core exec time of the (A, B) program via TimelineSim,
    minimized over a small PREP1_MS sweep (the virtual-time slot for pair
    1's prep interacts with the class's work size). Returns ns; caches
    (ns, best_prep1_ms). Falls back to a load heuristic without the sim."""
    key = (A, Bx)
    if key not in _TIME_CACHE:
        try:
            from concourse.timeline_sim import TimelineSim

            best = None
            for ms in (0.012, 0.02):
                for c1 in (
                    None, (22, 22, 14, 4, 2), (18, 18, 16, 6, 4, 2)
                ):
                    for km in (True, False):
                        for mm_ in (True, False):
                            nc = build_nc(A, Bx, prep1_ms=ms, ch1=c1,
                                          kdma_merge=km, mask_mid=mm_)
                            t = float(TimelineSim(nc, trace=False).simulate())
                            if best is None or t < best[0]:
                                best = (t, ms, c1, km, mm_)
            _TIME_CACHE[key] = best
        except Exception:
            _TIME_CACHE[key] = (53.3 * (A + Bx) + 10.0 * Bx, None, None, True, True)
    return _TIME_CACHE[key][0]


def best_cfg(A, Bx):
    _class_time(A, Bx)
    return _TIME_CACHE[(A, Bx)][1:]


def best_prep1_ms(A, Bx):
    return best_cfg(A, Bx)[0]


def plan(vl):
    """Partition 32 batches into 8 cores x (pair0, pair1) and <= MAXPROGS
    program classes. Returns list of classes:
      {"T0", "T1", "cores": [list of 4 original batch indices per core]}
    Cores across classes are disjoint and cover all batches. Candidate
    partitions are scored by the max TimelineSim-modeled class time.
    """
    vl = np.asarray(vl).reshape(-1).astype(np.int64)
    assert vl.shape[0] == B
    order = np.argsort(-vl, kind="stable")
    pairs = [(int(order[2 * i]), int(order[2 * i + 1])) for i in range(B // 2)]
    # pair extent = max valid_len, rounded up to 8 (fp32r ISA granularity)
    ext = [min(K, (int(vl[p[0]]) + 7) // 8 * 8) for p in pairs]

    npair = len(pairs)          # 16
    ncore = npair // 2          # 8
    slot0 = list(range(ncore))            # pair indices 0..7  (big)
    slot1 = list(range(ncore, npair))     # pair indices 8..15 (small)

    # enumerate candidate partitions; prefilter by load to bound the number
    # of distinct (A, B) programs that get TimelineSim-scored
    cands = []
    for m in range(1, min(MAXPROGS, ncore) + 1):
        for comp in _compositions(ncore, m):
            # class k slot0 group: contiguous slice of slot0 pairs (desc);
            # class k slot1 group: contiguous slice of slot1 pairs, assigned
            # in REVERSE class order so the largest-extent class gets the
            # smallest slot1 extents.
            s0_groups = []
            off = 0
            for nk in comp:
                s0_groups.append(slot0[off : off + nk])
                off += nk
            s1_groups = [None] * m
            off = 0
            for k in reversed(range(m)):
                nk = comp[k]
                s1_groups[k] = slot1[off : off + nk]
                off += nk
            classes = []
            loads = []
            for k in range(m):
                A = max(ext[i] for i in s0_groups[k])
                Bx = max(ext[i] for i in s1_groups[k])
                A, Bx = max(A, Bx), min(A, Bx)
                loads.append(A + Bx)
                classes.append((A, Bx, s0_groups[k], s1_groups[k]))
            cands.append((max(loads), sum(loads), m, classes))
    cands.sort(key=lambda c: c[:3])
    best_load = cands[0][0]
    # keep partitions within 48 of the best max-load; sim-score those
    cands = [c for c in cands if c[0] <= best_load + 96][:40]
    best = None
    for _, _, m, classes in cands:
        t = max(_class_time(A, Bx) for A, Bx, _, _ in classes)
        score = (t, sum(_class_time(A, Bx) for A, Bx, _, _ in classes), m)
        if best is None or score < best[0]:
            best = (score, classes)

    _, classes = best
    out = []
    for A, Bx, a_pairs, b_pairs in classes:
        cores = []
        for i in range(len(a_pairs)):
            pa = pairs[a_pairs[i]]
            pb = pairs[b_pairs[len(b_pairs) - 1 - i]]
            cores.append([pa[0], pa[1], pb[0], pb[1]])
        out.append({"T0": int(A), "T1": int(Bx), "cores": cores})
    return out


def get_nc(T0, T1):
    ms, c1, km, mm_ = best_cfg(T0, T1)
    key = (T0, T1, ms, c1, km, mm_, os.environ.get("MM_RELAXED", "1"),
           os.environ.get("CH0", ""), os.environ.get("CH1", ""),
           os.environ.get("FBUFS", ""), os.environ.get("EPI_AFTER", ""))
    if key not in _NC_CACHE:
        _NC_CACHE[key] = build_nc(T0, T1, prep1_ms=ms, ch1=c1, kdma_merge=km,
                                  mask_mid=mm_)
    return _NC_CACHE[key]


def kernel(queries, keys, values, valid_lens, Wq, Wk, wv):
    from concourse.bass_utils import run_bass_kernel_spmd

    queries = np.ascontiguousarray(queries, dtype=np.float32)
    keys = np.ascontiguousarray(keys, dtype=np.float32)
    values = np.ascontiguousarray(values, dtype=np.float32)
    vl = np.ascontiguousarray(np.asarray(valid_lens).reshape(B), dtype=np.int32)
    Wq = np.ascontiguousarray(Wq, dtype=np.float32)
    Wk = np.ascontiguousarray(Wk, dtype=np.float32)
    wv2 = np.ascontiguousarray(wv, dtype=np.float32).reshape(H, 1)

    out = np.empty((B, Q, V), dtype=np.float32)
    for cls in plan(vl):
        nc = get_nc(cls["T0"], cls["T1"])
        in_maps = []
        for bidx in cls["cores"]:
            in_maps.append(
                {
                    "queries": queries[bidx],
                    "keys": keys[bidx],
                    "values": values[bidx],
                    "valid_lens": vl[bidx].reshape(BPC, 1),
                    "Wq": Wq,
                    "Wk": Wk,
                    "wv": wv2,
                }
            )
        res = run_bass_kernel_spmd(nc, in_maps, core_ids=list(range(len(in_maps))))
        for bidx, r in zip(cls["cores"], res.results):
            out[bidx] = r["out"]
    return out


if __name__ == "__main__":
    rng = np.random.default_rng(0)
    q = rng.standard_normal((B, Q, H), dtype=np.float32)
    k = rng.standard_normal((B, K, H), dtype=np.float32)
    v = rng.standard_normal((B, K, V), dtype=np.float32)
    vl = rng.integers(1, K + 1, size=(B,)).astype(np.int32)
    Wq = rng.standard_normal((H, H), dtype=np.float32) / np.sqrt(H)
    Wk = rng.standard_normal((H, H), dtype=np.float32) / np.sqrt(H)
    wv = rng.standard_normal((H,), dtype=np.float32) / np.sqrt(H)
    out = kernel(queries=q, keys=k, values=v, valid_lens=vl, Wq=Wq, Wk=Wk, wv=wv)
    print(out.shape, out.dtype, np.abs(out).mean())


# revision 71
# speedup vs baseline: 1.0018x; 1.0013x over previous
"""Additive (Bahdanau) attention on 8 Trainium2 NeuronCores.

Full inputs in, full output out. Data-parallel over batch with
input-value-specialized programs: the masked softmax makes key positions
k >= valid_len irrelevant (they get -1e9 before exp), so each compiled
program only computes feats/tanh up to a per-pair key extent T (rounded
to 8 -- fp32r matmuls need an even moving free count on real HW, and
f32r with a partial-partition destination is rejected outright, hence
plain f32 for the projection / output matmuls). Batches are sorted by
valid_len, paired (2 batches pack the 128 SBUF partitions: rows 0-63 =
batch even's 64 hidden dims, 64-127 = batch odd), and pairs are
assigned to cores big-with-small. Cores are grouped into <= MAXPROGS
classes; each class compiles one program parameterized by (T0, T1) =
slot extents (T0 >= T1 so the serial epilogue hangs off the small
pair). Candidate partitions are scored by TimelineSim (max modeled
class time, minimized also over the PREP1_MS virtual-schedule knob).

The feats pipeline runs in bf16 by default (BFMIN=600): tanh cost on
ACT is dtype-independent, but DVE adds get the 4x perf mode and PE
matmuls run 1 cyc/row at any width (f32r only reaches that at moving
width >= 256). Output error ~2e-3 absmax-relative vs the 2e-2 gate.

Per (batch b, query j):  scores[j, k] = sum_h wv[h] * tanh(qW[b,j,h] + kW[b,k,h])
Layout on chip: h on partitions, k on free dim.
  - DVE tensor_scalar_add broadcasts qW[:, j] onto kW (2x fp32 / 4x bf16)
  - ACT does one big in-place tanh over a j-chunk at a time (the
    bottleneck engine: ~0.83ns/elem/lane, no fast dtype mode)
  - PE contracts with wv via a sliding-window stationary matrix,
    accumulating all 64 j's into one [128, T] PSUM bank; the additive
    -1e9 mask matmul CLOSES the group (start comes from j=0) so the
    valid_lens DMA stays off the startup critical path
  - ACT Exp (PSUM->SBUF) with fused accum_out row-sum, DVE reciprocal
  - PE transposes attn, accumulated matmul vs values, scale rows by
    1/sum, DMA out. Pair 0's exp/epilogue is interleaved into pair 1's
    main loop so ACT never stalls on PE's matmul tail.

Scheduling constraint: matmul/transpose instructions lower to a fused
LDWEIGHTS (S3_LW) slot that fits only ONE semaphore wait. So every PE
operand's last writer is kept on DVE or ACT (per-engine sems merge into
one monotonic wait); DMA-landed constants get a small DVE copy first,
and one priming PE op observes the DVE clock early.
"""

import os
import sys

import numpy as np

for _p in ("/opt/trn_rl_repo", "/root/.axon_site/_ro/trn_rl_repo"):
    if os.path.isdir(_p) and _p not in sys.path:
        sys.path.append(_p)

B, Q, K, H, V = 32, 64, 512, 64, 64
NCORES = 8
BPC = B // NCORES          # batches per core
NPAIR = BPC // 2           # batch pairs per core
NEG = -1e9
MAXPROGS = int(os.environ.get("MAXPROGS", "8"))

_NC_CACHE = {}


def _chunks(total, role):
    """j-chunk schedule summing to `total` (=Q).

    Head taper: chunk i+1's DVE adds must finish inside chunk i's tanh
    plus the accumulated slack, or ACT stalls. With adds ~0.8x tanh per j
    the sizes can grow ~1.25x per step from a small start.
    """
    env = os.environ.get("CH0" if role == "head" else "CH1")
    if env:
        c = [int(x) for x in env.split(",")]
    elif role == "head":
        c = [2, 4, 8, 16, 17, 17]
    else:  # tail: big first, tiny last so the epilogue starts asap
        c = [16, 16, 12, 8, 6, 4, 2]
    assert sum(c) == total, c
    return c


def build_nc(T0=512, T1=512, mm_relaxed=None, prep1_ms=None, act2=None,
             ch1=None, ch0=None, kdma_merge=True, mask_mid=True):
    """One-core program for pair extents (T0, T1), T0 >= T1."""
    if mm_relaxed is None:
        mm_relaxed = os.environ.get("MM_RELAXED", "1") == "1"
    if prep1_ms is None:
        prep1_ms = float(os.environ.get("PREP1_MS", "0.02"))
    if act2 is None:
        act2 = os.environ.get("ACT2", "0") == "1"
    import concourse.bass as bass  # noqa: F401
    from concourse import mybir
    from concourse import tile
    from concourse.masks import make_identity

    f32 = mybir.dt.float32
    f32r = mybir.dt.float32r if mm_relaxed else mybir.dt.float32
    bf16 = mybir.dt.bfloat16
    i32 = mybir.dt.int32
    Tanh = mybir.ActivationFunctionType.Tanh
    Exp = mybir.ActivationFunctionType.Exp
    Copy = mybir.ActivationFunctionType.Copy

    from concourse import bacc

    nc = bacc.Bacc("TRN2", target_bir_lowering=False, debug=False, num_devices=NCORES)

    queries_d = nc.dram_tensor("queries", [BPC, Q, H], f32, kind="ExternalInput")
    keys_d = nc.dram_tensor("keys", [BPC, K, H], f32, kind="ExternalInput")
    values_d = nc.dram_tensor("values", [BPC, K, V], f32, kind="ExternalInput")
    vlens_d = nc.dram_tensor("valid_lens", [BPC, 1], i32, kind="ExternalInput")
    Wq_d = nc.dram_tensor("Wq", [H, H], f32, kind="ExternalInput")
    Wk_d = nc.dram_tensor("Wk", [H, H], f32, kind="ExternalInput")
    wv_d = nc.dram_tensor("wv", [H, 1], f32r, kind="ExternalInput")
    out_d = nc.dram_tensor("out", [BPC, Q, V], f32, kind="ExternalOutput")

    # fp32r matmuls require an even moving free count (and 8B-aligned
    # patterns); round extents to multiples of 8.
    Ts = [min(K, (int(t) + 7) // 8 * 8) for t in (T0, T1)]
    assert Ts[0] >= Ts[1] >= 8 and Ts[0] <= K
    # bf16 feats pipeline: required for T < 256 (f32r matmuls are 4x slower
    # below 256 moving width); optional above it (BFMIN lowers the cutoff)
    BFMIN = int(os.environ.get("BFMIN", "600"))
    BF = [t < BFMIN or t < 256 for t in Ts]
    CKs = [(t + 127) // 128 for t in Ts]    # 128-wide key chunks loaded/used
    CHUNKS_BY_PAIR = [
        list(ch0) if ch0 is not None else _chunks(Q, "head"),
        list(ch1) if ch1 is not None else _chunks(Q, "tail"),
    ]
    FMAXs = [max(c) for c in CHUNKS_BY_PAIR]
    # F pools must fit SBUF (192KB/partition) alongside ~40KB of other
    # tiles; shrink the ring when the extents are large.
    fbytes = sum(
        FMAXs[p] * Ts[p] * (2 if BF[p] else 4) for p in range(NPAIR)
    )
    FBUFS = int(os.environ.get("FBUFS", "6"))
    FBUFS = max(2, min(FBUFS, (150 * 1024) // max(1, fbytes)))

    with tile.TileContext(nc) as tc:
        with (
            tc.tile_pool(name="consts", bufs=1) as consts,
            tc.tile_pool(name="work", bufs=2) as work,
            tc.tile_pool(name="feats", bufs=FBUFS) as fpool,
            tc.tile_pool(name="soft", bufs=2) as soft,
            tc.tile_pool(name="ps", bufs=2, space="PSUM") as ps,
        ):
            # ---- input DMAs: trigger order = priority. SP queue carries the
            # critical keys/queries (HWDGE generation is a single serialized
            # ~630ns/DMA resource, so order matters); ACT queue takes the
            # weights + vl; DVE stays DMA-free for compute.
            # Full-K loads merge both batches into ONE DMA (one 625ns HWDGE
            # generation instead of two; the extra unread rows cost only
            # ~180ns of transfer when CK < 4).
            # merge pays (one 625ns gen saved vs extra rows transferred)
            # only when the pair already loads >= 3 chunks
            KMERGE = [kdma_merge and c >= 3 for c in CKs]
            KL = [K // 128 if m else c for c, m in zip(CKs, KMERGE)]
            keys_pair = []
            for p in range(NPAIR):
                kp = consts.tile([128, 2, KL[p], H], f32, name=f"keys_pair{p}")
                keys_pair.append(kp)

            def keys_dma(p):
                if KMERGE[p]:
                    nc.sync.dma_start(
                        out=keys_pair[p],
                        in_=keys_d[2 * p : 2 * p + 2].rearrange(
                            "b (c p) h -> p b c h", p=128
                        ),
                    )
                else:
                    for half in range(2):
                        nc.sync.dma_start(
                            out=keys_pair[p][:, half, :, :],
                            in_=keys_d[2 * p + half, 0 : CKs[p] * 128, :].rearrange(
                                "(c p) h -> p c h", p=128
                            ),
                        )

            keys_dma(0)
            # Wk gates the kT projection (the longest prep chain) -> first
            # on the ACT queue; Wq (query projection) next; vl last.
            Wk_sb = consts.tile([H, H], f32)
            nc.scalar.dma_start(out=Wk_sb, in_=Wk_d[:, :])
            Wq_sb = consts.tile([H, H], f32)
            nc.scalar.dma_start(out=Wq_sb, in_=Wq_d[:, :])
            q_all = consts.tile([Q, BPC, H], f32)
            nc.sync.dma_start(out=q_all, in_=queries_d.rearrange("b q h -> q b h"))
            vl_i = consts.tile([2, NPAIR, 1], i32)
            nc.scalar.dma_start(
                out=vl_i, in_=vlens_d.rearrange("(c r) o -> r c o", r=2)
            )

            keys_dma(1)

            # Sliding-window stationary matrix for the wv contraction:
            # wvmat[r, 64] = wv[r] for r < 64 ; wvmat[64+r, 128] = wv[r].
            # lhsT for q-index j is wvmat[:, 64-j : 192-j].
            wvmat0 = consts.tile([128, 256], f32r)
            nc.vector.memset(wvmat0.bitcast(f32), 0.0)
            nc.sync.dma_start(out=wvmat0[0:64, 64:65], in_=wv_d[:, :])
            nc.sync.dma_start(out=wvmat0[64:128, 128:129], in_=wv_d[:, :])

            # Row-selector for the additive-mask matmul.
            sel20 = consts.tile([2, 128], f32r)
            nc.vector.memset(sel20.bitcast(f32), 0.0)
            nc.vector.memset(sel20[0:1, 0:64].bitcast(f32), 1.0)
            # DVE can't write at partition base 1; stage row 1 and DMA it over.
            sel2row = consts.tile([1, 128], f32r)
            nc.vector.memset(sel2row.bitcast(f32), 0.0)
            nc.vector.memset(sel2row[0:1, 64:128].bitcast(f32), 1.0)
            nc.sync.dma_start(out=sel20[1:2, :], in_=sel2row)

            # ---- values: off the critical path, loaded per pair ----
            v0s = []
            for p in range(NPAIR):
                v0 = work.tile([128, 2, KL[p], V], f32, tag=f"v0_{p}", bufs=1)
                if KMERGE[p]:
                    nc.sync.dma_start(
                        out=v0,
                        in_=values_d[2 * p : 2 * p + 2].rearrange(
                            "b (c p) v -> p b c v", p=128
                        ),
                    )
                else:
                    for half in range(2):
                        nc.sync.dma_start(
                            out=v0[:, half, :, :],
                            in_=values_d[2 * p + half, 0 : CKs[p] * 128, :].rearrange(
                                "(c p) v -> p c v", p=128
                            ),
                        )
                v0s.append(v0)

            # ---- constants (DVE-finalized so PE waits merge) ----
            ident0 = consts.tile([128, 128], f32)
            make_identity(nc, ident0)
            ident = consts.tile([128, 128], f32)
            nc.vector.tensor_copy(ident, ident0)

            Wq2 = consts.tile([H, H], bf16)
            nc.vector.tensor_copy(Wq2, Wq_sb)
            Wk2 = consts.tile([H, H], bf16)
            nc.vector.tensor_copy(Wk2, Wk_sb)

            # small consts finalized on the (otherwise idle) Pool engine so
            # the DVE stream stays clear for the prep copies + first adds;
            # PE pays a single mergeable Pool-sem wait for them.
            iota2_i = consts.tile([2, 512], i32)
            nc.gpsimd.iota(iota2_i, pattern=[[1, 512]], base=0, channel_multiplier=0)
            iota2 = consts.tile([2, 512], f32)
            nc.gpsimd.tensor_copy(iota2, iota2_i)

            vl_f = consts.tile([2, NPAIR, 1], f32)
            nc.gpsimd.tensor_copy(vl_f, vl_i)

            wvmat = consts.tile([128, 256], f32r)
            nc.gpsimd.tensor_copy(wvmat, wvmat0)
            sel2 = consts.tile([2, 128], f32r)
            nc.gpsimd.tensor_copy(sel2, sel20)
            if any(BF):
                wvmat_bf = consts.tile([128, 256], bf16)
                nc.gpsimd.tensor_copy(wvmat_bf, wvmat0)
                sel2_bf = consts.tile([2, 128], bf16)
                nc.gpsimd.tensor_copy(sel2_bf, sel20)

            # Priming PE op: observe the DVE clock once so constant operands
            # never cost later matmuls a second wait slot.
            prime_ps = ps.tile([32, 32], f32, tag="tail", bufs=2)
            nc.tensor.transpose(prime_ps, ident[0:32, 0:32], ident[0:32, 0:32])
            # keep PE's busy-streak alive until the keys arrive so the first
            # projection matmuls run at the warm clock
            for _w in range(int(os.environ.get("WARMUP", "6"))):
                wps = ps.tile([128, 128], f32, tag="tail", bufs=2, name=f"wps{_w}")
                nc.tensor.transpose(wps, ident, ident)

            # ---- per-pair long-lived tiles ----
            kT_pair = []    # [128(h2), T(k)]
            qWT_pair = []   # [128(h2), 64(q)]
            amask = []      # [2, T]
            vals_pair = []  # [128, 2, CK, V]
            for p in range(NPAIR):
                fdt = bf16 if BF[p] else f32r
                kT_pair.append(consts.tile([128, Ts[p]], fdt, name=f"kT_pair{p}"))
                qWT_pair.append(consts.tile([128, Q], f32, name=f"qWT_pair{p}"))
                amask.append(
                    consts.tile([2, Ts[p]], bf16 if BF[p] else f32r, name=f"amask{p}")
                )
                vals_pair.append(
                    consts.tile([128, 2, CKs[p], V], bf16, name=f"vals{p}")
                )

            # ---- prep: ordered so the serial chain keys-DMA -> transpose ->
            # copy -> projection -> kT copy -> adds is as short as possible.
            # keysT half1's PSUM->SBUF copy runs on ACT (idle until the first
            # tanh) in parallel with DVE's half0 copy.
            def prep_pair(p):
                T, CK = Ts[p], CKs[p]
                keysT_pss = []
                for half in range(2):
                    keysT_ps = ps.tile(
                        [H, 512], f32, tag="prep", bufs=4,
                        name=f"keysT_ps{2*p+half}",
                    )
                    for c in range(CK):
                        nc.tensor.transpose(
                            keysT_ps[:, 128 * c : 128 * (c + 1)],
                            keys_pair[p][:, half, c, :],
                            ident,
                        )
                    keysT_pss.append(keysT_ps)
                qT_pss = []
                for half in range(2):
                    b = 2 * p + half
                    qT_ps = ps.tile([H, Q], f32, tag="prep", bufs=4, name=f"qT_ps{b}")
                    nc.tensor.transpose(qT_ps, q_all[:, b, :], ident[0:Q, 0:Q])
                    qT_pss.append(qT_ps)
                # PSUM->SBUF copies: for pair 0 BOTH keysT copies run on ACT
                # (idle until the first tanh) so DVE's FIFO is free for the
                # qT/qWT/kT copies that gate the first adds; pair 1's run on
                # DVE (mid-kernel, ACT is the bottleneck stream there)
                keysT_sb0 = work.tile([H, CK * 128], bf16, tag="keysT_sb", bufs=4)
                if p == 0 and act2:
                    nc.scalar.activation(
                        keysT_sb0, keysT_pss[0][:, 0 : CK * 128], Copy
                    )
                else:
                    nc.vector.tensor_copy(keysT_sb0, keysT_pss[0][:, 0 : CK * 128])
                qT_sbs = []
                for half in range(2):
                    qT_sb = work.tile([H, Q], bf16, tag="qT_sb", bufs=4)
                    nc.vector.tensor_copy(qT_sb, qT_pss[half])
                    qT_sbs.append(qT_sb)
                keysT_sb1 = work.tile([H, CK * 128], bf16, tag="keysT_sb", bufs=4)
                if p == 0:
                    # parallel with DVE's half-0 copy on the startup chain
                    nc.scalar.activation(
                        keysT_sb1, keysT_pss[1][:, 0 : CK * 128], Copy
                    )
                else:
                    # mid-kernel: ACT is the bottleneck stream, DVE has slack
                    nc.vector.tensor_copy(keysT_sb1, keysT_pss[1][:, 0 : CK * 128])
                keysT_sbs = [keysT_sb0, keysT_sb1]
                # projections, col-tiled so both halves land in one [128, *]
                # PSUM tile (half 1 via tile_position=(0, 64))
                qWT_ps = ps.tile([128, Q], f32, tag="prep", bufs=4, name=f"qWT_ps{p}")
                for half in range(2):
                    rows = slice(64 * half, 64 * half + 64)
                    nc.tensor.matmul(
                        qWT_ps[rows, :], lhsT=Wq2, rhs=qT_sbs[half],
                        start=True, stop=True,
                        tile_position=(0, 64 * half),
                    )
                kT_ps = ps.tile([128, 512], f32, tag="prep", bufs=4, name=f"kT_ps{p}")
                for half in range(2):
                    rows = slice(64 * half, 64 * half + 64)
                    nc.tensor.matmul(
                        kT_ps[rows, 0:T], lhsT=Wk2, rhs=keysT_sbs[half][:, 0:T],
                        start=True, stop=True,
                        tile_position=(0, 64 * half),
                    )
                if p == 0:
                    # ACT is idle before the first tanh; doing the qWT copy
                    # there lets DVE start the (bigger) kT copy immediately
                    nc.scalar.activation(qWT_pair[p], qWT_ps, Copy)
                else:
                    nc.vector.tensor_copy(qWT_pair[p], qWT_ps)
                if p == 0 and os.environ.get("KTACT", "0") == "1":
                    nc.scalar.activation(kT_pair[p], kT_ps[:, 0:T], Copy)
                else:
                    nc.vector.tensor_copy(kT_pair[p], kT_ps[:, 0:T])
                return kT_ps

            def late_mask(p):
                # mask row: needed only by the group-closing matmul (Pool)
                nc.gpsimd.tensor_scalar(
                    out=amask[p],
                    in0=iota2[:, 0 : Ts[p]],
                    scalar1=vl_f[:, p, :],
                    scalar2=NEG,
                    op0=mybir.AluOpType.is_ge,
                    op1=mybir.AluOpType.mult,
                )

            def late_vals(p):
                # values copy: needed only by the epilogue (Pool)
                src_v = (
                    v0s[p][:, :, 0 : CKs[p], :] if KL[p] > CKs[p] else v0s[p]
                )
                nc.gpsimd.tensor_copy(vals_pair[p], src_v)

            # ---- main: scores -> softmax -> output ----
            def main_pair(p, interleave=(), epilogues=(), kT_ps_early=None):
                T, CK = Ts[p], CKs[p]
                CHUNKS = CHUNKS_BY_PAIR[p]
                FMAX = FMAXs[p]
                fdt = bf16 if BF[p] else f32r
                wvm = wvmat_bf if BF[p] else wvmat
                sel = sel2_bf if BF[p] else sel2
                scores_ps = ps.tile([128, 512], f32, tag=f"scores{p}", bufs=1)
                j0 = 0
                ci = 0
                for csz in CHUNKS:
                    F = fpool.tile([128, FMAX, T], fdt, tag=f"F{p}", bufs=FBUFS)
                    # chunk 0 reads kT straight from PSUM (full-rate DVE, but
                    # ~0.6us earlier than waiting for the SBUF copy)
                    kT_src = (
                        kT_ps_early[:, 0:T] if (ci == 0 and kT_ps_early is not None)
                        else kT_pair[p]
                    )
                    for jj in range(csz):
                        j = j0 + jj
                        nc.vector.tensor_scalar_add(
                            out=F[:, jj, :],
                            in0=kT_src,
                            scalar1=qWT_pair[p][:, j : j + 1],
                        )
                    nc.scalar.activation(F[:, 0:csz, :], F[:, 0:csz, :], Tanh)
                    for jj in range(csz):
                        j = j0 + jj
                        nc.tensor.matmul(
                            scores_ps[:, 0:T],
                            lhsT=wvm[:, 64 - j : 192 - j],
                            rhs=F[:, jj, :],
                            start=(j == 0),
                            stop=(mask_mid and j == Q - 1),
                        )
                    j0 += csz
                    ci += 1
                    if mask_mid and ci == MASK_AFTER:
                        # additive mask joins MID-group (j=0 already opened
                        # it): -1e9 where k >= valid_len, so exp -> 0 there
                        # exactly. Off the serial close chain: after the
                        # final tanh only the last j-matmuls remain.
                        nc.tensor.matmul(
                            scores_ps[:, 0:T], lhsT=sel, rhs=amask[p],
                            start=False, stop=False,
                        )
                    for at, fn in interleave:
                        if ci == at:
                            fn()
                if not mask_mid:
                    # additive mask closes the accumulation group
                    nc.tensor.matmul(
                        scores_ps[:, 0:T], lhsT=sel, rhs=amask[p],
                        start=False, stop=True,
                    )
                for fn in epilogues:
                    fn()

                def epilogue():
                    attn_sb = soft.tile([128, T], f32, tag=f"attn{p}", bufs=1)
                    sumexp = soft.tile([128, 1], f32, tag=f"sumexp{p}", bufs=1)
                    nc.scalar.activation(
                        attn_sb, scores_ps[:, 0:T], Exp, accum_out=sumexp
                    )
                    recip = soft.tile([128, 1], f32, tag=f"recip{p}", bufs=1)
                    nc.vector.reciprocal(recip, sumexp)

                    attnT_sb = soft.tile([128, CK, 128], bf16, tag=f"attnT{p}", bufs=1)
                    attnT_ps = ps.tile([128, 4, 128], f32, tag="tail", bufs=2)
                    for c in range(CK):
                        cw = min(128, T - 128 * c)
                        nc.tensor.transpose(
                            attnT_ps[0:cw, c, :],
                            attn_sb[:, 128 * c : 128 * c + cw],
                            ident,
                        )
                    for c in range(CK):
                        cw = min(128, T - 128 * c)
                        nc.vector.tensor_copy(
                            attnT_sb[0:cw, c, :], attnT_ps[0:cw, c, :]
                        )

                    out_pair = soft.tile([128, V], f32, tag=f"out_pair{p}", bufs=1)
                    out_ps = ps.tile([128, V], f32, tag="tail", bufs=2)
                    for half in range(2):
                        rows = slice(64 * half, 64 * half + 64)
                        for c in range(CK):
                            cw = min(128, T - 128 * c)
                            nc.tensor.matmul(
                                out_ps[rows, :],
                                lhsT=attnT_sb[0:cw, c, rows],
                                rhs=vals_pair[p][0:cw, half, c, :],
                                start=(c == 0),
                                stop=(c == CK - 1),
                                tile_position=(0, 64 * half),
                            )
                    nc.vector.tensor_scalar_mul(out=out_pair, in0=out_ps, scalar1=recip)
                    nc.sync.dma_start(
                        out=out_d[2 * p : 2 * p + 2].rearrange("b q v -> (b q) v"),
                        in_=out_pair,
                    )

                return epilogue

            EPI_AFTER = int(os.environ.get("EPI_AFTER", "2"))
            MASK_AFTER = int(os.environ.get("MASK_AFTER", "2"))
            VALS_AFTER = int(os.environ.get("VALS_AFTER", "5"))
            PREP1_AFTER = int(os.environ.get("PREP1_AFTER", "2"))
            PREP1_MS = prep1_ms

            def prep_pair1():
                # optionally push pair 1's prep later in the scheduler's
                # virtual timeline so it can't displace pair 0's first adds
                # in the in-order DVE stream
                if PREP1_MS > 0:
                    with tc.tile_wait_until(PREP1_MS):
                        prep_pair(1)
                else:
                    prep_pair(1)

            kT_ps0 = prep_pair(0)
            # pair 1's prep is issued a couple of chunks into main 0 so the
            # scheduler can't hoist its transposes/copies into pair 0's
            # serial prep -> first-tanh chain.
            epi0 = main_pair(
                0,
                interleave=[
                    (1, lambda: late_mask(0)),
                    (PREP1_AFTER, prep_pair1),
                    (VALS_AFTER, lambda: late_vals(0)),
                ],
                kT_ps_early=kT_ps0 if os.environ.get("KTPS", "0") == "1" else None,
            )
            epi1 = main_pair(
                1,
                interleave=[
                    (1, lambda: late_mask(1)),
                    (2, lambda: late_vals(1)),
                    (EPI_AFTER, epi0),
                ],
            )
            epi1()

    nc.compile()
    return nc


def _compositions(n, m):
    """All ways to write n as ordered sum of m positive ints."""
    if m == 1:
        yield (n,)
        return
    for first in range(1, n - m + 2):
        for rest in _compositions(n - first, m - 1):
            yield (first,) + rest


_TIME_CACHE = {}


def _class_time(A, Bx):
    """Modeled per-core exec time of the (A, B) program via TimelineSim,
    minimized over a small PREP1_MS sweep (the virtual-time slot for pair
    1's prep interacts with the class's work size). Returns ns; caches
    (ns, best_prep1_ms). Falls back to a load heuristic without the sim."""
    key = (A, Bx)
    if key not in _TIME_CACHE:
        try:
            from concourse.timeline_sim import TimelineSim

            best = None
            for ms in (0.012, 0.02):
                for c1 in (
                    None, (22, 22, 14, 4, 2), (18, 18, 16, 6, 4, 2),
                    (24, 20, 12, 6, 2),
                ):
                    for km in (True, False):
                        for mm_ in (True, False):
                            nc = build_nc(A, Bx, prep1_ms=ms, ch1=c1,
                                          kdma_merge=km, mask_mid=mm_)
                            t = float(TimelineSim(nc, trace=False).simulate())
                            if best is None or t < best[0]:
                                best = (t, ms, c1, km, mm_)
            _TIME_CACHE[key] = best
        except Exception:
            _TIME_CACHE[key] = (53.3 * (A + Bx) + 10.0 * Bx, None, None, True, True)
    return _TIME_CACHE[key][0]


def best_cfg(A, Bx):
    _class_time(A, Bx)
    return _TIME_CACHE[(A, Bx)][1:]


def best_prep1_ms(A, Bx):
    return best_cfg(A, Bx)[0]


def plan(vl):
    """Partition 32 batches into 8 cores x (pair0, pair1) and <= MAXPROGS
    program classes. Returns list of classes:
      {"T0", "T1", "cores": [list of 4 original batch indices per core]}
    Cores across classes are disjoint and cover all batches. Candidate
    partitions are scored by the max TimelineSim-modeled class time.
    """
    vl = np.asarray(vl).reshape(-1).astype(np.int64)
    assert vl.shape[0] == B
    order = np.argsort(-vl, kind="stable")
    pairs = [(int(order[2 * i]), int(order[2 * i + 1])) for i in range(B // 2)]
    # pair extent = max valid_len, rounded up to 8 (fp32r ISA granularity)
    ext = [min(K, (int(vl[p[0]]) + 7) // 8 * 8) for p in pairs]

    npair = len(pairs)          # 16
    ncore = npair // 2          # 8
    slot0 = list(range(ncore))            # pair indices 0..7  (big)
    slot1 = list(range(ncore, npair))     # pair indices 8..15 (small)

    # enumerate candidate partitions; prefilter by load to bound the number
    # of distinct (A, B) programs that get TimelineSim-scored
    cands = []
    for m in range(1, min(MAXPROGS, ncore) + 1):
        for comp in _compositions(ncore, m):
            # class k slot0 group: contiguous slice of slot0 pairs (desc);
            # class k slot1 group: contiguous slice of slot1 pairs, assigned
            # in REVERSE class order so the largest-extent class gets the
            # smallest slot1 extents.
            s0_groups = []
            off = 0
            for nk in comp:
                s0_groups.append(slot0[off : off + nk])
                off += nk
            s1_groups = [None] * m
            off = 0
            for k in reversed(range(m)):
                nk = comp[k]
                s1_groups[k] = slot1[off : off + nk]
                off += nk
            classes = []
            loads = []
            for k in range(m):
                A = max(ext[i] for i in s0_groups[k])
                Bx = max(ext[i] for i in s1_groups[k])
                A, Bx = max(A, Bx), min(A, Bx)
                loads.append(A + Bx)
                classes.append((A, Bx, s0_groups[k], s1_groups[k]))
            cands.append((max(loads), sum(loads), m, classes))
    cands.sort(key=lambda c: c[:3])
    best_load = cands[0][0]
    # keep partitions within 48 of the best max-load; sim-score those
    cands = [c for c in cands if c[0] <= best_load + 96][:40]
    best = None
    for _, _, m, classes in cands:
        t = max(_class_time(A, Bx) for A, Bx, _, _ in classes)
        score = (t, sum(_class_time(A, Bx) for A, Bx, _, _ in classes), m)
        if best is None or score < best[0]:
            best = (score, classes)

    _, classes = best
    out = []
    for A, Bx, a_pairs, b_pairs in classes:
        cores = []
        for i in range(len(a_pairs)):
            pa = pairs[a_pairs[i]]
            pb = pairs[b_pairs[len(b_pairs) - 1 - i]]
            cores.append([pa[0], pa[1], pb[0], pb[1]])
        out.append({"T0": int(A), "T1": int(Bx), "cores": cores})
    return out


def get_nc(T0, T1):
    ms, c1, km, mm_ = best_cfg(T0, T1)
    key = (T0, T1, ms, c1, km, mm_, os.environ.get("MM_RELAXED", "1"),
           os.environ.get("CH0", ""), os.environ.get("CH1", ""),
           os.environ.get("FBUFS", ""), os.environ.get("EPI_AFTER", ""))
    if key not in _NC_CACHE:
        _NC_CACHE[key] = build_nc(T0, T1, prep1_ms=ms, ch1=c1, kdma_merge=km,
                                  mask_mid=mm_)
    return _NC_CACHE[key]


def kernel(queries, keys, values, valid_lens, Wq, Wk, wv):
    from concourse.bass_utils import run_bass_kernel_spmd

    queries = np.ascontiguousarray(queries, dtype=np.float32)
    keys = np.ascontiguousarray(keys, dtype=np.float32)
    values = np.ascontiguousarray(values, dtype=np.float32)
    vl = np.ascontiguousarray(np.asarray(valid_lens).reshape(B), dtype=np.int32)
    Wq = np.ascontiguousarray(Wq, dtype=np.float32)
    Wk = np.ascontiguousarray(Wk, dtype=np.float32)
    wv2 = np.ascontiguousarray(wv, dtype=np.float32).reshape(H, 1)

    out = np.empty((B, Q, V), dtype=np.float32)
    for cls in plan(vl):
        nc = get_nc(cls["T0"], cls["T1"])
        in_maps = []
        for bidx in cls["cores"]:
            in_maps.append(
                {
                    "queries": queries[bidx],
                    "keys": keys[bidx],
                    "values": values[bidx],
                    "valid_lens": vl[bidx].reshape(BPC, 1),
                    "Wq": Wq,
                    "Wk": Wk,
                    "wv": wv2,
                }
            )
        res = run_bass_kernel_spmd(nc, in_maps, core_ids=list(range(len(in_maps))))
        for bidx, r in zip(cls["cores"], res.results):
            out[bidx] = r["out"]
    return out


if __name__ == "__main__":
    rng = np.random.default_rng(0)
    q = rng.standard_normal((B, Q, H), dtype=np.float32)
    k = rng.standard_normal((B, K, H), dtype=np.float32)
    v = rng.standard_normal((B, K, V), dtype=np.float32)
    vl = rng.integers(1, K + 1, size=(B,)).astype(np.int32)
    Wq = rng.standard_normal((H, H), dtype=np.float32) / np.sqrt(H)
    Wk = rng.standard_normal((H, H), dtype=np.float32) / np.sqrt(H)
    wv = rng.standard_normal((H,), dtype=np.float32) / np.sqrt(H)
    out = kernel(queries=q, keys=k, values=v, valid_lens=vl, Wq=Wq, Wk=Wk, wv=wv)
    print(out.shape, out.dtype, np.abs(out).mean())
